# revision 52
# baseline (speedup 1.0000x reference)
"""MLA (multi-head latent attention) forward on 8 TRN2 NeuronCores.

Sharding: 2-way data-parallel over batch x 4-way tensor-parallel over heads.
Core c handles batch b=c//4 and heads 4g..4g+3 where g=c%4. Each core runs an
identical SPMD program on its shard; the host sums the 4 partial outputs per
batch (the o_proj contribution of each head group) and transposes.

Layout: activations are feature-major ([feature, token]) so every matmul
contracts over the partition dim; x is transposed once by the DMA XBAR.
Probabilities are computed transposed (s[tk, tq]) so softmax needs no
max-subtraction (scores are bounded ~6) and P@V contracts naturally;
denominators come from ones-matmuls + fast reciprocal + DRAM-bounce
partition-broadcast. RMSNorm scaling is per-token so it commutes with the
B-projections: both q and kv normalizations are applied at copy-out of the
projected tensors, keeping the whole norm pipeline off the TensorE stream.
Weights are pre-tiled on the host so every weight DMA is contiguous.
"""

import numpy as np
import ml_dtypes

B, T, HIDDEN = 2, 2048, 2048
NUM_HEADS = 16
QK_NOPE, QK_ROPE, HEAD_DIM, V_HEAD = 128, 64, 192, 128
KV_LORA, Q_LORA = 512, 1536
EPS = 1e-6
NCORES = 8
HPC = 4  # heads per core

KC = HIDDEN // 128
TT = T // 128
TQ = T // 512
NQ = Q_LORA // 128
NKV = (KV_LORA + HPC * QK_ROPE) // 128
NL = KV_LORA // 128

BF16 = ml_dtypes.bfloat16

_CACHE = {}


def _build():
    import concourse.bass as bass
    import concourse.tile as tile
    from concourse import bacc, mybir
    from concourse.bass import ts
    from concourse.masks import make_identity

    f32 = mybir.dt.float32
    bf = mybir.dt.bfloat16
    AF = mybir.ActivationFunctionType

    nc = bacc.Bacc(
        "TRN2",
        target_bir_lowering=False,
        debug=False,
        enable_asserts=True,
        num_devices=NCORES,
    )

    def din(name, shape, dt=bf):
        return nc.dram_tensor(name, shape, dt, kind="ExternalInput").ap()

    # weights pre-tiled on host: contiguous per-tile DMA loads
    x_ap = din("x", [T, HIDDEN])                      # [t, d] (XBAR-transposed)
    qaw_ap = din("qaw", [NQ, 128, KC, 128])           # per col-block [p, kk, c]
    kvaw_ap = din("kvaw", [NKV, 128, KC, 128])
    qbw_ap = din("qbw", [2, 128, NQ, 2 * HEAD_DIM])   # pair: [nope0|nope1|ropes]
    kvbw_ap = din("kvbw", [128, NL, HPC * (QK_NOPE + V_HEAD)])
    ow_ap = din("ow", [128, HPC, HIDDEN])
    mask_ap = din("mask", [128, 896])                 # 0/1 causal bank (bf16)
    ones128_ap = din("ones128", [128, 128])
    out_ap = nc.dram_tensor("out", [HIDDEN, T], bf, kind="ExternalOutput").ap()

    def eng(idx):
        return nc.scalar if idx % 2 else nc.vector

    def copy(e, out, in_):
        if e is nc.scalar:
            nc.scalar.copy(out, in_)
        else:
            nc.vector.tensor_copy(out, in_)

    with tile.TileContext(nc) as tc:
        with tc.tile_pool(name="consts", bufs=1) as consts, \
             tc.tile_pool(name="trans", bufs=3) as trans, \
             tc.tile_pool(name="dram", bufs=1, space="DRAM") as dram, \
             tc.tile_pool(name="act", bufs=1) as act:

            mask = consts.tile([128, 896], bf)
            nc.sync.dma_start(out=mask, in_=mask_ap)
            ones128 = consts.tile([128, 128], bf)
            nc.sync.dma_start(out=ones128, in_=ones128_ap)
            eps1 = consts.tile([128, 1], f32)
            nc.vector.memset(eps1, EPS)

            xq = act.tile([128, NQ, T], bf)
            xkv = act.tile([128, NKV, T], bf)
            rq_b = act.tile([128, T], f32)
            rkv_b = act.tile([128, T], f32)
            rkvT = act.tile([128, TT], f32)

            # ---- Stage A: xT via DMA-XBAR; xq = qaw.T@xT; xkv = kvaw.T@xT
            with tc.tile_pool(name="stageA", bufs=1) as pA, \
                 tc.tile_pool(name="wa", bufs=3) as pwa, \
                 tc.tile_pool(name="pB", bufs=1) as pB, \
                 tc.tile_pool(name="psumA", bufs=1, space="PSUM") as psumA:
                xT = pA.tile([128, KC, T], bf)
                for k in range(KC):
                    nc.sync.dma_start(
                        out=xT[:, k, :], in_=x_ap[:, ts(k, 128)], transpose=True
                    )

                for src_ap, ncols, dst in ((qaw_ap, NQ, xq), (kvaw_ap, NKV, xkv)):
                    for n in range(ncols):
                        wa = pwa.tile([128, KC, 128], bf, tag="wa", bufs=3)
                        nc.gpsimd.dma_start(out=wa, in_=src_ap[n])
                        for t in range(TQ):
                            psm = psumA.tile([128, 512], f32, tag="psm", bufs=4)
                            for kk in range(KC):
                                nc.tensor.matmul(
                                    out=psm,
                                    lhsT=wa[:, kk, :],
                                    rhs=xT[:, kk, ts(t, 512)],
                                    start=(kk == 0),
                                    stop=(kk == KC - 1),
                                )
                            nc.scalar.copy(dst[:, n, ts(t, 512)], psm)

                # ---- Stage B: M=128 ones-matmul gives sumsq pre-broadcast
                # across all partitions; rstd lands directly in rq_b/rkv_b.
                for t in range(TQ):
                    for src, nn, lora, rb in (
                        (xq, NQ, Q_LORA, rq_b),
                        (xkv, NL, KV_LORA, rkv_b),
                    ):
                        psd = psumA.tile([128, 512], f32, tag="psd", bufs=2)
                        for n in range(nn):
                            sq = pB.tile([128, 512], bf, tag="sq", bufs=13)
                            nc.vector.tensor_mul(
                                sq, src[:, n, ts(t, 512)], src[:, n, ts(t, 512)]
                            )
                            nc.tensor.matmul(
                                out=psd, lhsT=ones128, rhs=sq,
                                start=(n == 0), stop=(n == nn - 1),
                            )
                        tmp = pB.tile([128, 512], f32, tag="tmp", bufs=2)
                        nc.scalar.activation(
                            out=tmp, in_=psd, func=AF.Sqrt, bias=eps1,
                            scale=1.0 / lora,
                        )
                        nc.vector.reciprocal_approx_fast(
                            out=rb[:, ts(t, 512)], in_=tmp
                        )
                # transposed rstd_kv column view for the v row-scaling
                rkv_d = dram.tile([1, T], f32)
                nc.gpsimd.dma_start(out=rkv_d, in_=rkv_b[0:1, :])
                nc.gpsimd.dma_start(
                    out=rkvT, in_=rkv_d.rearrange("o (tt p) -> (o p) tt", p=128)
                )

            kv_lat = xkv  # raw; norm applied at projection copy-out

            # ---- Stages D/E/F
            with tc.tile_pool(name="att", bufs=1) as patt, \
                 tc.tile_pool(name="owp", bufs=1) as powp, \
                 tc.tile_pool(name="psumD", bufs=1, space="PSUM") as psumD:
                qn = patt.tile([128, HPC, T], bf)
                qr = patt.tile([128, HPC, T], bf)  # per head, other rows zero
                nc.gpsimd.memset(qr, 0.0)
                kn = patt.tile([128, HPC, T], bf)
                vv = patt.tile([128, TT, HPC * V_HEAD], bf)

                with tc.tile_pool(name="wD", bufs=1) as pw:
                    qbws = []
                    for pair in range(2):
                        qbw = pw.tile([128, NQ, 2 * HEAD_DIM], bf, tag="qbw",
                                      bufs=2, name=f"qbw{pair}")
                        nc.sync.dma_start(out=qbw, in_=qbw_ap[pair])
                        qbws.append(qbw)
                    kvbw = pw.tile([128, NL, HPC * (QK_NOPE + V_HEAD)], bf)
                    nc.sync.dma_start(out=kvbw, in_=kvbw_ap)
                    ow = powp.tile([128, HPC, HIDDEN], bf)
                    nc.sync.dma_start(out=ow, in_=ow_ap)

                    # ---- Stage D: q (x rstd_q), k_nope (x rstd_kv), v (x rstd_kv)
                    for pair in range(2):
                        qbw = qbws[pair]
                        for t in range(TQ):
                            for sub in range(3):  # nope0 | nope1 | rope pair
                                ps = psumD.tile([128, 512], f32, tag="psm", bufs=2)
                                for kk in range(NQ):
                                    nc.tensor.matmul(
                                        out=ps,
                                        lhsT=qbw[:, kk, ts(sub, 128)],
                                        rhs=xq[:, kk, ts(t, 512)],
                                        start=(kk == 0),
                                        stop=(kk == NQ - 1),
                                    )
                                if sub < 2:
                                    nc.vector.tensor_mul(
                                        qn[:, 2 * pair + sub, ts(t, 512)],
                                        ps, rq_b[:, ts(t, 512)],
                                    )
                                else:
                                    nc.vector.tensor_mul(
                                        qr[0:64, 2 * pair, ts(t, 512)],
                                        ps[0:64, :], rq_b[0:64, ts(t, 512)],
                                    )
                                    nc.vector.tensor_mul(
                                        qr[64:128, 2 * pair + 1, ts(t, 512)],
                                        ps[64:128, :], rq_b[64:128, ts(t, 512)],
                                    )
                    for h in range(HPC):
                        for t in range(TQ):
                            ps3 = psumD.tile([128, 512], f32, tag="psm", bufs=2)
                            for kk in range(NL):
                                nc.tensor.matmul(
                                    out=ps3,
                                    lhsT=kvbw[:, kk, ts(h, 256)][:, 0:128],
                                    rhs=kv_lat[:, kk, ts(t, 512)],
                                    start=(kk == 0),
                                    stop=(kk == NL - 1),
                                )
                            nc.vector.tensor_mul(
                                kn[:, h, ts(t, 512)], ps3, rkv_b[:, ts(t, 512)]
                            )
                    vcols = kvbw.rearrange(
                        "p kk (h two dv) -> p kk h two dv", h=HPC, two=2
                    )
                    for tt in range(TT):
                        psv = psumD.tile([128, 512], f32, tag="psm", bufs=2)
                        for kk in range(NL):
                            nc.tensor.matmul(
                                out=psv,
                                lhsT=kv_lat[:, kk, ts(tt, 128)],
                                rhs=vcols[:, kk, :, 1, :],
                                start=(kk == 0),
                                stop=(kk == NL - 1),
                            )
                        nc.scalar.mul(vv[:, tt, :], psv, mul=rkvT[:, tt:tt + 1])

                # ---- Stage E+F: causal attention; o_proj one chunk behind
                with tc.tile_pool(name="attn_i", bufs=2) as pai, \
                     tc.tile_pool(name="ob", bufs=2) as pob, \
                     tc.tile_pool(name="rdb", bufs=2) as prdb:
                    attn_tiles = []

                    def attention_chunk(i):
                        attn_i = pai.tile([128, HPC, 512], bf, tag="attn_i", bufs=2)
                        for h in range(HPC):
                            nj = 4 * i + 4
                            pso = psumD.tile([128, 512], f32, tag="pso", bufs=2)
                            psd = psumD.tile([128, 512], f32, tag="psd", bufs=1)

                            def consume_batch(batch, last):
                                for jc, exc in batch:
                                    nc.tensor.matmul(
                                        out=psd, lhsT=ones128, rhs=exc,
                                        start=(jc == 0),
                                        stop=(last and jc == batch[-1][0]),
                                    )
                                for jc, exc in batch:
                                    nc.tensor.matmul(
                                        out=pso, lhsT=vv[:, jc, ts(h, V_HEAD)],
                                        rhs=exc,
                                        start=(jc == 0),
                                        stop=(last and jc == batch[-1][0]),
                                    )

                            pending = []
                            for j in range(nj):
                                pss = psumD.tile([128, 512], f32, tag="pss", bufs=3)
                                nc.tensor.matmul(
                                    out=pss,
                                    lhsT=kn[:, h, ts(j, 128)],
                                    rhs=qn[:, h, ts(i, 512)],
                                    start=True,
                                    stop=False,
                                )
                                nc.tensor.matmul(
                                    out=pss,
                                    lhsT=xkv[:, NL + h // 2, ts(j, 128)],
                                    rhs=qr[:, h, ts(i, 512)],
                                    start=False,
                                    stop=True,
                                )
                                if len(pending) == 4:
                                    consume_batch(pending, False)
                                    pending = []
                                ex = trans.tile([128, 512], bf, tag="ex", bufs=6)
                                nc.scalar.activation(out=ex, in_=pss, func=AF.Exp)
                                off = j * 128 - i * 512
                                if off >= 0:
                                    nc.vector.tensor_mul(
                                        ex, ex, mask[:, 384 - off:896 - off]
                                    )
                                pending.append((j, ex))
                            if pending:
                                consume_batch(pending, True)

                            rdb = prdb.tile([128, 512], f32, tag="rdb", bufs=2)
                            nc.vector.reciprocal_approx_fast(out=rdb, in_=psd)
                            nc.vector.tensor_mul(attn_i[:, h, :], pso, rdb)
                        attn_tiles.append(attn_i)

                    def oproj_chunk(i):
                        attn_i = attn_tiles[i]
                        for m in range(TT):
                            psf = psumD.tile([128, 512], f32, tag="psm", bufs=2)
                            for kk in range(HPC):
                                nc.tensor.matmul(
                                    out=psf,
                                    lhsT=ow[:, kk, ts(m, 128)],
                                    rhs=attn_i[:, kk, :],
                                    start=(kk == 0),
                                    stop=(kk == HPC - 1),
                                )
                            ob = pob.tile([128, 512], bf, tag="ob", bufs=3)
                            if i == TQ - 1:
                                copy(eng(m), ob, psf)  # attention done: ACT free
                            else:
                                nc.vector.tensor_copy(ob, psf)
                            (nc.sync if m % 2 else nc.gpsimd).dma_start(
                                out=out_ap[ts(m, 128), ts(i, 512)], in_=ob
                            )

                    attention_chunk(0)
                    for i in range(1, TQ):
                        attention_chunk(i)
                        oproj_chunk(i - 1)
                    oproj_chunk(TQ - 1)

    nc.compile()
    return nc


def _tile_w(w):
    """[K, N] -> [N/128, 128, K/128, 128] so each col-block loads contiguously."""
    K, N = w.shape
    return np.ascontiguousarray(
        w.reshape(K // 128, 128, N // 128, 128).transpose(2, 1, 0, 3))


def _prep(inputs):
    x = np.asarray(inputs["hidden_states"], np.float32)
    qaw = np.asarray(inputs["q_a_w"], np.float32)
    qalw = np.asarray(inputs["q_a_ln_w"], np.float32)
    qbw = np.asarray(inputs["q_b_w"], np.float32)
    kvaw = np.asarray(inputs["kv_a_w"], np.float32)
    kvlw = np.asarray(inputs["kv_a_ln_w"], np.float32)
    kvbw = np.asarray(inputs["kv_b_w"], np.float32)
    ow = np.asarray(inputs["o_w"], np.float32)

    scale = 1.0 / np.sqrt(np.float32(HEAD_DIM))
    qbw_f = (qbw * qalw[:, None] * scale).astype(BF16)
    kvbw_f = (kvbw * kvlw[:, None]).astype(BF16)
    qaw_t = _tile_w(qaw.astype(BF16))               # [NQ, 128, KC, 128]

    r = np.arange(128)[:, None]
    j = np.arange(896)[None, :]
    mask = np.where((j - 384) >= r, 1.0, 0.0).astype(BF16)
    ones128 = np.ones((128, 128), BF16)

    def lat_tiled(w):  # [KV_LORA, N] -> [128, NL, N] (p, kk, n)
        return np.ascontiguousarray(w.reshape(NL, 128, -1).transpose(1, 0, 2))

    in_maps = []
    for c in range(NCORES):
        b, g = c // 4, c % 4
        qbw_g = qbw_f[:, g * HPC * HEAD_DIM:(g + 1) * HPC * HEAD_DIM]
        pairs = []
        for pair in range(HPC // 2):
            h0, h1 = 2 * pair, 2 * pair + 1
            cols = np.concatenate([
                qbw_g[:, h0 * HEAD_DIM:h0 * HEAD_DIM + QK_NOPE],
                qbw_g[:, h1 * HEAD_DIM:h1 * HEAD_DIM + QK_NOPE],
                qbw_g[:, h0 * HEAD_DIM + QK_NOPE:(h0 + 1) * HEAD_DIM],
                qbw_g[:, h1 * HEAD_DIM + QK_NOPE:(h1 + 1) * HEAD_DIM],
            ], axis=1)  # [Q_LORA, 384]
            pairs.append(cols.reshape(NQ, 128, 384).transpose(1, 0, 2))
        qbw_c = np.ascontiguousarray(np.stack(pairs))   # [2, 128, NQ, 384]

        kvaw_g = np.concatenate(
            [kvaw[:, :KV_LORA],
             kvaw[:, KV_LORA + g * HPC * QK_ROPE:
                  KV_LORA + (g + 1) * HPC * QK_ROPE]], axis=1).astype(BF16)

        in_maps.append({
            "x": x[b].astype(BF16),
            "qaw": qaw_t,
            "kvaw": _tile_w(kvaw_g),
            "qbw": qbw_c,
            "kvbw": lat_tiled(kvbw_f[:, g * HPC * 256:(g + 1) * HPC * 256]),
            "ow": np.ascontiguousarray(
                ow[g * HPC * V_HEAD:(g + 1) * HPC * V_HEAD]
                .astype(BF16).reshape(HPC, 128, HIDDEN).transpose(1, 0, 2)),
            "mask": mask,
            "ones128": ones128,
        })
    return in_maps


def _ensure_trace_shim():
    """This image lacks antenv.axon_hooks; synthesize it so a trace=True (or
    BASS_TRACE=1) invocation degrades gracefully instead of crashing."""
    import sys
    import types
    try:
        import antenv.axon_hooks  # noqa: F401
        return
    except Exception:
        pass
    try:
        import antenv
        import trn_agent_boot.trn_boot as tb
        hook = tb._ntff_profile_via_ctypes("/opt/axon/libaxon_pjrt.so")
        mod = types.ModuleType("antenv.axon_hooks")
        mod.get_axon_ntff_profile_hook = lambda: hook
        mod.set_axon_ntff_profile_hook = lambda h: None
        antenv.axon_hooks = mod
        sys.modules["antenv.axon_hooks"] = mod
        import concourse.bass_utils as bu
        bu.upload_artifacts = lambda tmpdir: tmpdir
    except Exception:
        pass


def kernel(**inputs):
    from concourse.bass_utils import run_bass_kernel_spmd

    _ensure_trace_shim()
    if "nc" not in _CACHE:
        _CACHE["nc"] = _build()
    nc = _CACHE["nc"]
    in_maps = _prep(inputs)
    try:
        res = run_bass_kernel_spmd(nc, in_maps, core_ids=list(range(NCORES)),
                                   **_CACHE.get("run_kwargs", {}))
    except Exception:
        # transient accelerator faults (e.g. NRT_EXEC_UNIT_UNRECOVERABLE) have
        # been observed after interrupted runs; one retry clears them
        import time
        time.sleep(2)
        res = run_bass_kernel_spmd(nc, in_maps, core_ids=list(range(NCORES)),
                                   **_CACHE.get("run_kwargs", {}))
    _CACHE["last_results"] = res
    out = np.zeros((B, T, HIDDEN), np.float32)
    for c in range(NCORES):
        out[c // 4] += np.asarray(res.results[c]["out"], np.float32).T
    return out
